# revision 32
# baseline (speedup 1.0000x reference)
"""Trainium2 Bass kernel for the 2-layer GATv2 + MLP-head model (nn_GAT_21028159881586).

Strategy (8 NeuronCores, SPMD single NEFF):
  * Destination-block partitioning: node -> (core, window-slot) assignment is
    LOAD-BALANCED on in-degree (LPT binning) so every one of the 240 windows
    has nearly the mean edge count -> minimal edge-tile padding T.
  * Per layer: node transforms on the local slice, one full AllGather of xl,
    then per destination window of 128 dst nodes:
      - self-loop edges form tile 0 of the window, loaded by a single
        contiguous HWDGE DMA from the core-local xl slice (no gather cost),
      - remaining edges: dma_gather of xl rows in (edge, channel) layout,
        split into 4 pieces <=512 idxs rotating over the 4 SWDGE queues
        (the random 512B reads are HBM-latency-bound; queue spread matters),
      - xr side via PE: dense fp8 scatter matrices s_T (dst x edge) and
        s_t (edge x dst); m = s_T.T @ xr + I @ gec in PSUM, Prelu on ACT,
      - L1 logits: lr * att_rep (DVE 2x) + halving tree per head,
      - L2 logits: signed att2 folded into Wl2/Wr2 columns on the host with
        channels permuted pos-att-first, so the Prelu stage splits into
        Prelu(.2) / scaled Prelu(5) column ranges and the logit is a plain
        halving-tree sum (no DVE multiply),
      - exp on ACT, replicated to all channels by an ACT broadcast-read exp
        (keeps the gw multiply in DVE 2x packed mode),
      - one PE matmul per 128-edge tile accumulates [agg | softmax-denom],
      - normalize (+ undo the att2 column scaling for L2), bias, ELU.
  * Layer-2 node transform pipelined per window, transposing h via SBUF-source
    DMA-transpose (no DRAM round-trip).
  * MLP head: batch rows are assigned to the core owning their var node.

fp16 data, fp32 PSUM accumulation.
"""

import heapq

import numpy as np

import concourse.bacc as bacc
import concourse.tile as tile
import concourse.mybir as mybir
from concourse.bass_utils import run_bass_kernel_spmd

fp8 = mybir.dt.float8e4

P = 128
NCORES = 8
N = 30000
WIN = 30
NLOC = WIN * P            # 3840
IN_DIM = 1281
KCH = 11
KPAD = KCH * P            # 1408
HID = 256
HEADS1 = 4
BLOC = 640
NEG = 0.2

f32 = mybir.dt.float32
f16 = mybir.dt.float16
i16 = mybir.dt.int16
AF = mybir.ActivationFunctionType
OP = mybir.AluOpType
AX = mybir.AxisListType

_nc_cache = {}


def _wrap16(idx2d: np.ndarray) -> np.ndarray:
    """(W, E) int -> (W*128, E//16) int16, wrapped in 16 partitions, replicated
    across the 8 gpsimd cores."""
    w, e = idx2d.shape
    assert e % 16 == 0
    t = idx2d.reshape(w, e // 16, 16).transpose(0, 2, 1)
    return np.tile(t, (1, 8, 1)).reshape(w * P, e // 16).astype(np.int16)


def _balance_nodes(deg: np.ndarray):
    """LPT binning of nodes into 240 (core, window) bins of <=128 nodes,
    balancing total in-degree per bin. Returns core_of_node, slot_of_node."""
    nbins = NCORES * WIN
    order = np.argsort(-deg, kind="stable")
    heap = [(0, b) for b in range(nbins)]
    heapq.heapify(heap)
    counts = np.zeros(nbins, np.int64)
    core_of = np.zeros(N, np.int64)
    slot_of = np.zeros(N, np.int64)
    for n in order:
        spill = []
        while True:
            s, b = heapq.heappop(heap)
            if counts[b] < P:
                break
            spill.append((s, b))
        for it in spill:
            heapq.heappush(heap, it)
        c, w = b // WIN, b % WIN
        core_of[n] = c
        slot_of[n] = w * P + counts[b]
        counts[b] += 1
        heapq.heappush(heap, (s + int(deg[n]), b))
    return core_of, slot_of


def _preprocess(inputs):
    x = np.asarray(inputs["x"], np.float32)
    ei = np.asarray(inputs["edge_index"]).astype(np.int64)
    var_idx = np.asarray(inputs["var_node_idx"]).astype(np.int64)
    wt = np.asarray(inputs["wt_onehot"], np.float32)
    mut = np.asarray(inputs["mut_onehot"], np.float32)

    # real edges only; the self loops become tile 0 of each window
    src_n = ei[0]
    dst_n = ei[1]
    deg = np.bincount(dst_n, minlength=N) + 1    # +1: self loop
    core_of_node, slot_of_node = _balance_nodes(deg)

    src_pad_all = core_of_node[src_n] * NLOC + slot_of_node[src_n]
    dcore = core_of_node[dst_n]
    dslot = slot_of_node[dst_n]

    order = np.argsort(dslot + dcore * NLOC, kind="stable")
    src_pad = src_pad_all[order]
    core_of = dcore[order]
    dloc = dslot[order]
    win_of = dloc // P

    flat = core_of * WIN + win_of
    counts = np.bincount(flat, minlength=NCORES * WIN)
    TG = int((counts.max() + P - 1) // P)        # gather tiles
    T = TG + 1                                   # + self tile
    ew = TG * P

    per_core = []
    for c in range(NCORES):
        sel = core_of == c
        sp_c, dl_c, w_c = src_pad[sel], dloc[sel], win_of[sel]
        srcw = np.zeros((WIN, ew), np.int64)     # padding gathers row 0
        drlw = np.full((WIN, T * P), -1, np.int64)   # -1 => padding edge
        # tile 0: self loops at slot position
        csel = core_of_node == c
        slots_c = slot_of_node[np.nonzero(csel)[0]]
        drlw[slots_c // P, slots_c % P] = slots_c % P
        for w in range(WIN):
            m = w_c == w
            k = int(m.sum())
            o = np.argsort(sp_c[m], kind="stable")   # HBM locality
            srcw[w, :k] = sp_c[m][o]
            drlw[w, P:P + k] = dl_c[m][o] - w * P
        si = _wrap16(srcw)                       # (WIN*P, ew//16) i16
        # s_t[w*128+p, t*128+d] = 1 if drl[w, t*128+p] == d  (edge-part, dst)
        dr_pt = drlw.reshape(WIN, T, P).transpose(0, 2, 1)     # [w, p, t]
        s_t = (dr_pt[:, :, :, None] == np.arange(P)[None, None, None, :])
        s_t = s_t.reshape(WIN * P, T * P)
        # s_T[w*128+d, t*128+e] = 1 if drl[w, t*128+e] == d   (dst-part, edge)
        dr_te = drlw.reshape(WIN, T, P)                        # [w, t, e]
        s_T = (np.arange(P)[None, :, None, None] == dr_te[:, None, :, :])
        s_T = s_T.reshape(WIN, P, T * P)
        per_core.append(dict(si=si,
                     s_t=s_t.astype(np.float32).astype(mybir.dt.np(fp8)),
                     s_T=s_T.reshape(WIN * P, T * P).astype(np.float32).astype(mybir.dt.np(fp8))))

    # ---- shared weights / constants
    def pad_kT(w, m):
        wp = np.zeros((KPAD, m), np.float32)
        wp[:IN_DIM] = w
        return wp.reshape(KCH, P, m).transpose(1, 0, 2).reshape(P, KCH * m).astype(np.float16)

    def two_chunk(w):
        m = w.shape[1]
        return w.reshape(2, P, m).transpose(1, 0, 2).reshape(P, 2 * m).astype(np.float16)

    # L1: att replicated to full window width (contiguous DVE multiply)
    att1 = np.asarray(inputs["att1"], np.float32)           # (4, 64)
    attrep1 = np.broadcast_to(np.tile(att1.reshape(1, HID), (1, T)),
                              (P, T * HID)).copy().astype(np.float16)

    # L2: fold signed att2 into the Wl2/Wr2 columns; permute pos-att first.
    att2 = np.asarray(inputs["att2"], np.float32).reshape(HID)
    perm2 = np.argsort(att2 < 0, kind="stable")             # pos/zero first
    n2pos = int((att2 >= 0).sum())
    a2p = att2[perm2]                                       # signed scales
    a2safe = np.where(np.abs(a2p) < 1e-12, 1.0, a2p)
    inva2 = (1.0 / a2safe).astype(np.float32)

    def rep_bias(b):
        return np.broadcast_to(np.asarray(b, np.float32)[None, :], (P, HID)).copy()

    wl2 = np.asarray(inputs["Wl2"], np.float32)[:, perm2] * a2p[None, :]
    wr2 = np.asarray(inputs["Wr2"], np.float32)[:, perm2] * a2p[None, :]
    bl2 = np.asarray(inputs["bl2"], np.float32)[perm2] * a2p
    br2 = np.asarray(inputs["br2"], np.float32)[perm2] * a2p
    bias2 = np.asarray(inputs["bias2"], np.float32)[perm2]

    hW1 = np.asarray(inputs["hW1"], np.float32).copy()
    hW1[0:HID] = hW1[0:HID][perm2]                          # permuted h2 rows
    wlr1 = np.concatenate([np.asarray(inputs["Wl1"], np.float32),
                           np.asarray(inputs["Wr1"], np.float32)], axis=1)
    wlr2 = np.concatenate([wl2, wr2], axis=1)
    shared = dict(
        wlr1=pad_kT(wlr1, 2 * HID),
        wlr2=two_chunk(wlr2),
        attrep1=attrep1,
        blr1=np.concatenate([rep_bias(inputs["bl1"]), rep_bias(inputs["br1"])], 1),
        bias1=rep_bias(inputs["bias1"]),
        blr2=np.concatenate([rep_bias(bl2), rep_bias(br2)], 1),
        bias2=rep_bias(bias2),
        inva2=np.broadcast_to(inva2[None, :], (P, HID)).copy(),
        hw1a=hW1[0:128].astype(np.float16),
        hw1b=hW1[128:256].astype(np.float16),
        hw1c=np.vstack([hW1[256:296], np.zeros((8, 128), np.float32)]).astype(np.float16),
        hw2=np.asarray(inputs["hW2"], np.float32).astype(np.float16),
        hw3=np.asarray(inputs["hW3"], np.float32).astype(np.float16),
        hb1=np.asarray(inputs["hb1"], np.float32).reshape(P, 1),
        hb2=np.asarray(inputs["hb2"], np.float32).reshape(64, 1),
        hb3=np.asarray(inputs["hb3"], np.float32).reshape(1, 1),
        ident=np.eye(P, dtype=np.float16),
    )

    # ---- per-core x slices, transposed + padded
    for c in range(NCORES):
        sel = core_of_node == c
        nodes = np.nonzero(sel)[0]
        slots = slot_of_node[nodes]
        xp = np.zeros((KPAD, NLOC), np.float32)
        xp[:IN_DIM, slots] = x[nodes].T
        per_core[c]["xt"] = xp.reshape(KCH, P, NLOC).transpose(1, 0, 2).reshape(
            P, KCH * NLOC).astype(np.float16)

    # ---- MLP batch assignment
    vcore = core_of_node[var_idx]
    vloc = slot_of_node[var_idx]
    batch_rows = []
    for c in range(NCORES):
        rows = np.nonzero(vcore == c)[0]
        assert len(rows) <= BLOC, f"core {c} has {len(rows)} batch rows > {BLOC}"
        batch_rows.append(rows)
        vi = np.zeros((1, BLOC), np.int64)
        vi[0, :len(rows)] = vloc[rows]
        per_core[c]["varloc"] = _wrap16(vi)
        wm = np.zeros((40, BLOC), np.float32)
        wm[:20, :len(rows)] = wt[rows].T
        wm[20:, :len(rows)] = mut[rows].T
        per_core[c]["wtmut"] = wm.astype(np.float16)

    return per_core, shared, batch_rows, (ew, n2pos)


def _build(key, no_collectives=False):
    ew, n2pos = key
    TG = ew // P
    T = TG + 1
    nc = bacc.Bacc("TRN2", target_bir_lowering=False, debug=False,
                   num_devices=1 if no_collectives else NCORES,
                   num_swdge_queues=4)

    io = {}
    io["xt"] = nc.dram_tensor("xt", [P, KCH * NLOC], f16, kind="ExternalInput")
    for nm, sh, dt in (
        ("wlr1", [P, KCH * 2 * HID], f16), ("wlr2", [P, 4 * HID], f16),
        ("attrep1", [P, T * HID], f16),
        ("blr1", [P, 2 * HID], f32), ("bias1", [P, HID], f32),
        ("blr2", [P, 2 * HID], f32), ("bias2", [P, HID], f32),
        ("inva2", [P, HID], f32),
        ("hw1a", [P, P], f16), ("hw1b", [P, P], f16), ("hw1c", [48, P], f16),
        ("hw2", [P, 64], f16), ("hw3", [64, 1], f16),
        ("hb1", [P, 1], f32), ("hb2", [64, 1], f32), ("hb3", [1, 1], f32),
        ("si", [WIN * P, ew // 16], i16),
        ("s_t", [WIN * P, T * P], fp8), ("s_T", [WIN * P, T * P], fp8),
        ("varloc", [P, BLOC // 16], i16), ("wtmut", [40, BLOC], f16),
        ("ident", [P, P], f16),
    ):
        io[nm] = nc.dram_tensor(nm, sh, dt, kind="ExternalInput")
    out = nc.dram_tensor("out", [1, BLOC], f32, kind="ExternalOutput")

    with tile.TileContext(nc) as tc:
        with (
            tc.tile_pool(name="const", bufs=1) as cp,
            tc.tile_pool(name="dram", bufs=1, space="DRAM") as dr,
        ):
            c_ = {}
            for nm in ("wlr2", "attrep1", "bias1", "blr2", "bias2", "inva2",
                       "hw1a", "hw1b", "hw1c", "hw2",
                       "hw3", "hb1", "hb2", "hb3", "varloc", "wtmut", "ident"):
                h = io[nm]
                c_[nm] = cp.tile(list(h.shape), h.dtype, tag=nm, name=f"c_{nm}")
                nc.sync.dma_start(c_[nm][:], h[:])

            xl1_loc = dr.tile([NLOC, HID], f16, name="xl1_loc")
            xr1_loc = dr.tile([NLOC, HID], f16)
            xl1_all = dr.tile([NLOC * NCORES, HID], f16, addr_space="Shared",
                              name="xl1_all")
            xl2_loc = dr.tile([NLOC, HID], f16, name="xl2_loc")
            xr2_loc = dr.tile([NLOC, HID], f16)
            xl2_all = dr.tile([NLOC * NCORES, HID], f16, addr_space="Shared",
                              name="xl2_all")
            h2_loc = dr.tile([NLOC, HID], f16)

            # ---------- phase A: layer-1 node transform ----------
            with (
                tc.tile_pool(name="pa_sb", bufs=2) as sb,
                tc.tile_pool(name="pa_xt", bufs=1) as xp,
                tc.tile_pool(name="pa_ps", bufs=4, space="PSUM") as ps,
            ):
                xt = xp.tile([P, KCH, NLOC], f16)
                xtv = io["xt"][:].rearrange("p (k n) -> p k n", k=KCH)
                for xq in range(4):
                    n0, n1 = xq * (NLOC // 4), (xq + 1) * (NLOC // 4)
                    nc.sync.dma_start(xt[:, :, n0:n1], xtv[:, :, n0:n1])
                wlr1 = xp.tile([P, KCH, 2 * HID], f16)
                nc.sync.dma_start(wlr1[:], io["wlr1"][:].rearrange("p (k n) -> p k n", k=KCH))
                blr1 = xp.tile([P, 2 * HID], f32)
                nc.sync.dma_start(blr1[:], io["blr1"][:])
                for nt in range(WIN):
                    pa = ps.tile([P, 2 * HID], f32, tag="pa")
                    for k in range(KCH):
                        nc.tensor.matmul(pa[:], lhsT=xt[:, k, nt * P:(nt + 1) * P],
                                         rhs=wlr1[:, k, :],
                                         start=(k == 0), stop=(k == KCH - 1))
                    o = sb.tile([P, 2 * HID], f16, tag="pao")
                    nc.vector.tensor_tensor(out=o[:], in0=pa[:], in1=blr1[:],
                                            op=OP.add)
                    rr = nt * P
                    nc.scalar.dma_start(xl1_loc[rr:rr + P, :], o[:, 0:HID])
                    nc.scalar.dma_start(xr1_loc[rr:rr + P, :], o[:, HID:2 * HID])

            nc.gpsimd.collective_compute(
                "AllGather", OP.bypass, replica_groups=[list(range(NCORES))],
                ins=[xl1_loc[:].opt()], outs=[xl1_all[:].opt()])

            # layer-1 message passing with the layer-2 node transform
            # pipelined per window (phase B)
            with (
                tc.tile_pool(name="pb_sb", bufs=2) as pbsb,
                tc.tile_pool(name="pb_ht", bufs=2) as pbhp,
                tc.tile_pool(name="pb_ps", bufs=2, space="PSUM") as pbps,
            ):
                blr2 = c_["blr2"]

                def post_l1(nt, h_t):
                    ht = pbhp.tile([P, 2, P], f16, tag="ht")
                    for k in range(2):
                        nc.sync.dma_start_transpose(
                            ht[:, k, :], h_t[:, k * P:(k + 1) * P])
                    pa = pbps.tile([P, 2 * HID], f32, tag="pb")
                    for k in range(2):
                        nc.tensor.matmul(
                            pa[:], lhsT=ht[:, k, :],
                            rhs=c_["wlr2"][:, k * 2 * HID:(k + 1) * 2 * HID],
                            start=(k == 0), stop=(k == 1))
                    o = pbsb.tile([P, 2 * HID], f16, tag="pbo")
                    nc.vector.tensor_tensor(out=o[:], in0=pa[:], in1=blr2[:],
                                            op=OP.add)
                    rr = nt * P
                    nc.scalar.dma_start(xl2_loc[rr:rr + P, :], o[:, 0:HID])
                    nc.scalar.dma_start(xr2_loc[rr:rr + P, :], o[:, HID:2 * HID])

                _emit_layer(nc, tc, ew=ew, heads=HEADS1, xl_all=xl1_all,
                            xl_loc=xl1_loc, xr_loc=xr1_loc, h_out=None,
                            attrep=c_["attrep1"], bias_mat=c_["bias1"],
                            inva=None, n2pos=None, io=io, ident=c_["ident"],
                            tag="l1", post_window=post_l1)

            nc.gpsimd.collective_compute(
                "AllGather", OP.bypass, replica_groups=[list(range(NCORES))],
                ins=[xl2_loc[:].opt()], outs=[xl2_all[:].opt()])

            _emit_layer(nc, tc, ew=ew, heads=1, xl_all=xl2_all,
                        xl_loc=xl2_loc, xr_loc=xr2_loc, h_out=h2_loc,
                        attrep=None, bias_mat=c_["bias2"], inva=c_["inva2"],
                        n2pos=n2pos, io=io, ident=c_["ident"], tag="l2")

            # ---------- MLP head ----------
            with (
                tc.tile_pool(name="mlp_sb", bufs=2) as sb,
                tc.tile_pool(name="mlp_ps", bufs=2, space="PSUM") as ps,
            ):
                sel = sb.tile([P, 2, BLOC], f16)
                nc.gpsimd.dma_gather(sel[:], h2_loc[:], c_["varloc"][:],
                                     num_idxs=BLOC, num_idxs_reg=BLOC,
                                     elem_size=HID, transpose=True)
                for c0, cn in ((0, 512), (512, BLOC - 512)):
                    z1p = ps.tile([P, 512], f32, tag="z1p")
                    nc.tensor.matmul(z1p[:, :cn], lhsT=c_["hw1a"][:],
                                     rhs=sel[:, 0, c0:c0 + cn], start=True, stop=False)
                    nc.tensor.matmul(z1p[:, :cn], lhsT=c_["hw1b"][:],
                                     rhs=sel[:, 1, c0:c0 + cn], start=False, stop=False)
                    nc.tensor.matmul(z1p[:, :cn], lhsT=c_["hw1c"][0:40, :],
                                     rhs=c_["wtmut"][:, c0:c0 + cn], start=False, stop=True)
                    z1 = sb.tile([P, 512], f16, tag="z1")
                    nc.scalar.activation(z1[:, :cn], z1p[:, :cn], AF.Relu,
                                         bias=c_["hb1"][:])
                    z2p = ps.tile([64, 512], f32, tag="z2p")
                    nc.tensor.matmul(z2p[:, :cn], lhsT=c_["hw2"][:],
                                     rhs=z1[:, :cn], start=True, stop=True)
                    z2 = sb.tile([64, 512], f16, tag="z2")
                    nc.scalar.activation(z2[:, :cn], z2p[:, :cn], AF.Relu,
                                         bias=c_["hb2"][:])
                    z3p = ps.tile([1, 512], f32, tag="z3p")
                    nc.tensor.matmul(z3p[:, :cn], lhsT=c_["hw3"][:],
                                     rhs=z2[:, :cn], start=True, stop=True)
                    z3 = sb.tile([1, 512], f32, tag="z3")
                    nc.scalar.activation(z3[:, :cn], z3p[:, :cn], AF.Identity,
                                         bias=c_["hb3"][:])
                    nc.sync.dma_start(out[0:1, c0:c0 + cn], z3[:, :cn])

    nc.compile()
    return nc


def _emit_layer(nc, tc, *, ew, heads, xl_all, xl_loc, xr_loc, h_out, attrep,
                bias_mat, inva, n2pos, io, ident, tag, post_window=None):
    TG = ew // P
    T = TG + 1
    CW = HID // heads
    NB = (T + 3) // 4
    # gather tiles split into 4 pieces <= 4 tiles (512 idxs) each
    npieces = (TG + 3) // 4
    bnds = [TG * i // npieces for i in range(npieces + 1)]
    pieces = [(a, b) for a, b in zip(bnds, bnds[1:]) if b > a]
    r_q = {}
    for tb, te in pieces:
        n = (te - tb) * P
        if n not in r_q:
            r_q[n] = nc.gpsimd.to_reg(n)
    with (
        tc.tile_pool(name=f"{tag}_g", bufs=6) as gp,
        tc.tile_pool(name=f"{tag}_s", bufs=3) as sp,
        tc.tile_pool(name=f"{tag}_si", bufs=8) as sip,
        tc.tile_pool(name=f"{tag}_lr", bufs=3) as lrp,
        tc.tile_pool(name=f"{tag}_gw", bufs=3) as gwp,
        tc.tile_pool(name=f"{tag}_wr", bufs=3) as wrp,
        tc.tile_pool(name=f"{tag}_e", bufs=3) as ep,
        tc.tile_pool(name=f"{tag}_pm", bufs=2 if post_window else 3,
                     space="PSUM") as pmp,
        tc.tile_pool(name=f"{tag}_pa", bufs=2, space="PSUM") as pap,
    ):
        for w in range(WIN):
            rows = slice(w * P, (w + 1) * P)
            si = sip.tile([P, ew // 16], i16, tag="si")
            nc.sync.dma_start(si[:], io["si"][rows, :])
            sT = sp.tile([P, T, P], fp8, tag="sT")
            nc.sync.dma_start(sT[:], io["s_T"][rows, :].rearrange("p (t e) -> p t e", t=T))
            st = sp.tile([P, T, P], fp8, tag="st")
            nc.sync.dma_start(st[:], io["s_t"][rows, :].rearrange("p (t e) -> p t e", t=T))
            xrw = sp.tile([P, HID], f16, tag="xrw")
            nc.sync.dma_start(xrw[:], xr_loc[rows, :])

            # tile 0 = self loops: contiguous local rows, plain DMA
            gec = gp.tile([P, T, HID], f16, tag="gec")
            nc.sync.dma_start(gec[:, 0, :], xl_loc[rows, :])
            for j, (tb, te) in enumerate(pieces):
                n = (te - tb) * P
                nc.gpsimd.dma_gather(gec[:, 1 + tb:1 + te, :], xl_all[:],
                                     si[:, tb * 8:te * 8],
                                     num_idxs=n, num_idxs_reg=r_q[n],
                                     elem_size=HID, transpose=False,
                                     single_packet=False,
                                     queue_num=(w + j) % 4)

            # m = xl[src] + xr[dst] in PSUM (xr via s_T matmul, xl via identity
            # matmul); lr = leaky_relu(m)   (edge, channel)
            lr = lrp.tile([P, T, HID], f16, tag="lr")
            for b in range(NB):
                nb = min(4, T - 4 * b)
                pm = pmp.tile([P, 4, HID], f32, tag="pm")
                for tt in range(nb):
                    t = 4 * b + tt
                    nc.tensor.matmul(pm[:, tt, :], lhsT=sT[:, t, :], rhs=xrw[:],
                                     start=True, stop=False)
                    nc.tensor.matmul(pm[:, tt, :], lhsT=ident[:],
                                     rhs=gec[:, t, :], start=False, stop=True)
                if n2pos is None:
                    nc.scalar.activation(lr[:, 4 * b:4 * b + nb, :], pm[:, 0:nb, :],
                                         AF.Prelu, alpha=NEG)
                else:
                    # signed att2 folded into the channels: pos-att columns
                    # get Prelu(.2); neg-att columns get .2*Prelu(5).
                    if n2pos > 0:
                        nc.scalar.activation(lr[:, 4 * b:4 * b + nb, 0:n2pos],
                                             pm[:, 0:nb, 0:n2pos],
                                             AF.Prelu, alpha=NEG)
                    if n2pos < HID:
                        nc.scalar.activation(lr[:, 4 * b:4 * b + nb, n2pos:HID],
                                             pm[:, 0:nb, n2pos:HID],
                                             AF.Prelu, alpha=1.0 / NEG, scale=NEG)

            # logits per head: tree-sum of lr * att over each head's channel
            # block (L1); for L2 the att weights are folded in already.
            lra = lr[:].rearrange("p t (h c) -> p t h c", h=heads)
            if attrep is not None:
                nc.vector.tensor_tensor(
                    out=lr[:].rearrange("p t c -> p (t c)"),
                    in0=lr[:].rearrange("p t c -> p (t c)"),
                    in1=attrep[:], op=OP.mult)
            wdt = CW
            while wdt > 1:
                half = wdt // 2
                nc.vector.tensor_tensor(
                    out=lra[:, :, :, 0:half], in0=lra[:, :, :, 0:half],
                    in1=lra[:, :, :, half:wdt], op=OP.add)
                wdt = half

            # exp(logit) -> gwx tail (denominator) + replicated to all CW
            # channels via ACT broadcast-read (keeps gw-mult in DVE 2x mode)
            gwx = gwp.tile([P, T, HID + heads], f16, tag="gwx")
            wrep = wrp.tile([P, T, HID], f16, tag="wrep")
            wre4 = wrep[:].rearrange("p t (h c) -> p t h c", h=heads)
            nc.scalar.activation(
                gwx[:, :, HID:HID + heads].rearrange("p t (h o) -> p t h o", o=1),
                lra[:, :, :, 0:1], AF.Exp)
            nc.scalar.activation(
                wre4[:], lra[:, :, :, 0:1].to_broadcast([P, T, heads, CW]),
                AF.Exp)
            nc.vector.tensor_tensor(
                out=gwx[:, :, 0:HID], in0=gec[:].rearrange("p t c -> p (t c)"),
                in1=wrep[:].rearrange("p t c -> p (t c)"), op=OP.mult)

            agg = pap.tile([P, HID + heads], f32, tag="agg")
            for t in range(T):
                nc.tensor.matmul(agg[:], lhsT=st[:, t, :], rhs=gwx[:, t, :],
                                 start=(t == 0), stop=(t == T - 1))

            # normalize + bias + ELU
            den = ep.tile([P, heads], f32, tag="den")
            nc.vector.tensor_scalar_add(den[:], agg[:, HID:HID + heads], 1e-16)
            rden = ep.tile([P, heads], f32, tag="rden")
            nc.vector.reciprocal(rden[:], den[:])
            hb = ep.tile([P, HID], f32, tag="hb")
            if inva is not None:
                # hb = (agg * rden) * inva  (rden is a per-partition scalar)
                nc.vector.scalar_tensor_tensor(out=hb[:], in0=agg[:, 0:HID],
                                               scalar=rden[:, 0:1], in1=inva[:],
                                               op0=OP.mult, op1=OP.mult)
            else:
                nc.vector.tensor_tensor(
                    out=hb[:].rearrange("p (h c) -> p h c", h=heads),
                    in0=agg[:, 0:HID].rearrange("p (h c) -> p h c", h=heads),
                    in1=rden[:].to_broadcast([P, heads, CW]), op=OP.mult)
            nc.vector.tensor_tensor(out=hb[:], in0=hb[:], in1=bias_mat[:], op=OP.add)
            # ELU: hb + r + exp(-r) - 1  with r = relu(-hb)
            r = ep.tile([P, HID], f32, tag="r")
            nc.scalar.activation(r[:], hb[:], AF.Relu, scale=-1.0)
            ex = ep.tile([P, HID], f32, tag="ex")
            nc.scalar.activation(ex[:], r[:], AF.Exp, scale=-1.0)
            t1 = ep.tile([P, HID], f32, tag="t1")
            nc.vector.tensor_tensor(out=t1[:], in0=hb[:], in1=r[:], op=OP.add)
            h_t = ep.tile([P, HID], f16, tag="h_t")
            nc.vector.scalar_tensor_tensor(out=h_t[:], in0=ex[:], scalar=-1.0,
                                           in1=t1[:], op0=OP.add, op1=OP.add)
            if h_out is not None:
                nc.scalar.dma_start(h_out[rows, :], h_t[:])
            if post_window is not None:
                post_window(w, h_t)


def kernel(**inputs):
    per_core, shared, batch_rows, key = _preprocess(inputs)

    if key not in _nc_cache:
        _nc_cache[key] = _build(key)
    nc = _nc_cache[key]

    in_maps = []
    for c in range(NCORES):
        m = dict(shared)
        m.update(per_core[c])
        in_maps.append({k: np.ascontiguousarray(v) for k, v in m.items()})

    res = run_bass_kernel_spmd(nc, in_maps, core_ids=list(range(NCORES)))

    B = len(np.asarray(inputs["var_node_idx"]))
    out = np.zeros((B,), np.float32)
    for c in range(NCORES):
        rows = batch_rows[c]
        out[rows] = res.results[c]["out"][0, :len(rows)]
    return out


# revision 34
# speedup vs baseline: 1.0477x; 1.0477x over previous
"""Trainium2 Bass kernel for the 2-layer GATv2 + MLP-head model (nn_GAT_21028159881586).

Strategy (8 NeuronCores, SPMD single NEFF):
  * Destination-block partitioning: node -> (core, window-slot) assignment is
    LOAD-BALANCED on in-degree (LPT binning) so every one of the 240 windows
    has nearly the mean edge count -> minimal edge-tile padding T.
  * Per layer: node transforms on the local slice, one full AllGather of xl,
    then per destination window of 128 dst nodes:
      - self-loop edges form tile 0 of the window, loaded by a single
        contiguous HWDGE DMA from the core-local xl slice (no gather cost),
      - remaining edges: dma_gather of xl rows in (edge, channel) layout,
        split into 4 pieces <=512 idxs rotating over the 4 SWDGE queues
        (the random 512B reads are HBM-latency-bound; queue spread matters),
      - xr side via PE: dense fp8 scatter matrices s_T (dst x edge) and
        s_t (edge x dst); m = s_T.T @ xr + I @ gec in PSUM, Prelu on ACT,
      - L1 logits: lr * att_rep (DVE 2x) + halving tree per head,
      - L2 logits: signed att2 folded into Wl2/Wr2 columns on the host with
        channels permuted pos-att-first, so the Prelu stage splits into
        Prelu(.2) / scaled Prelu(5) column ranges and the logit is a plain
        halving-tree sum (no DVE multiply),
      - exp on ACT, replicated to all channels by an ACT broadcast-read exp
        (keeps the gw multiply in DVE 2x packed mode),
      - one PE matmul per 128-edge tile accumulates [agg | softmax-denom],
      - normalize (+ undo the att2 column scaling for L2), bias, ELU.
  * Layer-2 node transform pipelined per window, transposing h via SBUF-source
    DMA-transpose (no DRAM round-trip).
  * MLP head: batch rows are assigned to the core owning their var node.

fp16 data, fp32 PSUM accumulation.
"""

import heapq

import numpy as np

import concourse.bacc as bacc
import concourse.tile as tile
import concourse.mybir as mybir
from concourse.bass_utils import run_bass_kernel_spmd

fp8 = mybir.dt.float8e4

P = 128
NCORES = 8
N = 30000
WIN = 30
NLOC = WIN * P            # 3840
IN_DIM = 1281
KCH = 11
KPAD = KCH * P            # 1408
HID = 256
HEADS1 = 4
BLOC = 640
NEG = 0.2

f32 = mybir.dt.float32
f16 = mybir.dt.float16
i16 = mybir.dt.int16
AF = mybir.ActivationFunctionType
OP = mybir.AluOpType
AX = mybir.AxisListType

_nc_cache = {}


def _wrap16(idx2d: np.ndarray) -> np.ndarray:
    """(W, E) int -> (W*128, E//16) int16, wrapped in 16 partitions, replicated
    across the 8 gpsimd cores."""
    w, e = idx2d.shape
    assert e % 16 == 0
    t = idx2d.reshape(w, e // 16, 16).transpose(0, 2, 1)
    return np.tile(t, (1, 8, 1)).reshape(w * P, e // 16).astype(np.int16)


def _balance_nodes(deg: np.ndarray):
    """LPT binning of nodes into 240 (core, window) bins of <=128 nodes,
    balancing total in-degree per bin. Returns core_of_node, slot_of_node."""
    nbins = NCORES * WIN
    order = np.argsort(-deg, kind="stable")
    heap = [(0, b) for b in range(nbins)]
    heapq.heapify(heap)
    counts = np.zeros(nbins, np.int64)
    core_of = np.zeros(N, np.int64)
    slot_of = np.zeros(N, np.int64)
    for n in order:
        spill = []
        while True:
            s, b = heapq.heappop(heap)
            if counts[b] < P:
                break
            spill.append((s, b))
        for it in spill:
            heapq.heappush(heap, it)
        c, w = b // WIN, b % WIN
        core_of[n] = c
        slot_of[n] = w * P + counts[b]
        counts[b] += 1
        heapq.heappush(heap, (s + int(deg[n]), b))
    return core_of, slot_of


def _preprocess(inputs):
    x = np.asarray(inputs["x"], np.float32)
    ei = np.asarray(inputs["edge_index"]).astype(np.int64)
    var_idx = np.asarray(inputs["var_node_idx"]).astype(np.int64)
    wt = np.asarray(inputs["wt_onehot"], np.float32)
    mut = np.asarray(inputs["mut_onehot"], np.float32)

    # real edges only; the self loops become tile 0 of each window
    src_n = ei[0]
    dst_n = ei[1]
    deg = np.bincount(dst_n, minlength=N) + 1    # +1: self loop
    core_of_node, slot_of_node = _balance_nodes(deg)

    src_pad_all = core_of_node[src_n] * NLOC + slot_of_node[src_n]
    dcore = core_of_node[dst_n]
    dslot = slot_of_node[dst_n]

    order = np.argsort(dslot + dcore * NLOC, kind="stable")
    src_pad = src_pad_all[order]
    core_of = dcore[order]
    dloc = dslot[order]
    win_of = dloc // P

    flat = core_of * WIN + win_of
    counts = np.bincount(flat, minlength=NCORES * WIN)
    TG = int((counts.max() + P - 1) // P)        # gather tiles
    T = TG + 1                                   # + self tile
    ew = TG * P

    per_core = []
    for c in range(NCORES):
        sel = core_of == c
        sp_c, dl_c, w_c = src_pad[sel], dloc[sel], win_of[sel]
        srcw = np.zeros((WIN, ew), np.int64)     # padding gathers row 0
        drlw = np.full((WIN, T * P), -1, np.int64)   # -1 => padding edge
        # tile 0: self loops at slot position
        csel = core_of_node == c
        slots_c = slot_of_node[np.nonzero(csel)[0]]
        drlw[slots_c // P, slots_c % P] = slots_c % P
        for w in range(WIN):
            m = w_c == w
            k = int(m.sum())
            o = np.argsort(sp_c[m], kind="stable")   # HBM locality
            srcw[w, :k] = sp_c[m][o]
            drlw[w, P:P + k] = dl_c[m][o] - w * P
        si = _wrap16(srcw)                       # (WIN*P, ew//16) i16
        # s_t[w*128+p, t*128+d] = 1 if drl[w, t*128+p] == d  (edge-part, dst)
        dr_pt = drlw.reshape(WIN, T, P).transpose(0, 2, 1)     # [w, p, t]
        s_t = (dr_pt[:, :, :, None] == np.arange(P)[None, None, None, :])
        s_t = s_t.reshape(WIN * P, T * P)
        # s_T[w*128+d, t*128+e] = 1 if drl[w, t*128+e] == d   (dst-part, edge)
        dr_te = drlw.reshape(WIN, T, P)                        # [w, t, e]
        s_T = (np.arange(P)[None, :, None, None] == dr_te[:, None, :, :])
        s_T = s_T.reshape(WIN, P, T * P)
        per_core.append(dict(si=si,
                     s_t=s_t.astype(np.float32).astype(mybir.dt.np(fp8)),
                     s_T=s_T.reshape(WIN * P, T * P).astype(np.float32).astype(mybir.dt.np(fp8))))

    # ---- shared weights / constants
    def pad_kT(w, m):
        wp = np.zeros((KPAD, m), np.float32)
        wp[:IN_DIM] = w
        return wp.reshape(KCH, P, m).transpose(1, 0, 2).reshape(P, KCH * m).astype(np.float16)

    def two_chunk(w):
        m = w.shape[1]
        return w.reshape(2, P, m).transpose(1, 0, 2).reshape(P, 2 * m).astype(np.float16)

    # L1: att replicated to full window width (contiguous DVE multiply)
    att1 = np.asarray(inputs["att1"], np.float32)           # (4, 64)
    attrep1 = np.broadcast_to(np.tile(att1.reshape(1, HID), (1, T)),
                              (P, T * HID)).copy().astype(np.float16)

    # L2: fold signed att2 into the Wl2/Wr2 columns; permute pos-att first.
    att2 = np.asarray(inputs["att2"], np.float32).reshape(HID)
    perm2 = np.argsort(att2 < 0, kind="stable")             # pos/zero first
    n2pos = int((att2 >= 0).sum())
    a2p = att2[perm2]                                       # signed scales
    a2safe = np.where(np.abs(a2p) < 1e-12, 1.0, a2p)
    inva2 = (1.0 / a2safe).astype(np.float32)

    def rep_bias(b):
        return np.broadcast_to(np.asarray(b, np.float32)[None, :], (P, HID)).copy()

    wl2 = np.asarray(inputs["Wl2"], np.float32)[:, perm2] * a2p[None, :]
    wr2 = np.asarray(inputs["Wr2"], np.float32)[:, perm2] * a2p[None, :]
    bl2 = np.asarray(inputs["bl2"], np.float32)[perm2] * a2p
    br2 = np.asarray(inputs["br2"], np.float32)[perm2] * a2p
    bias2 = np.asarray(inputs["bias2"], np.float32)[perm2]

    hW1 = np.asarray(inputs["hW1"], np.float32).copy()
    hW1[0:HID] = hW1[0:HID][perm2]                          # permuted h2 rows
    wlr1 = np.concatenate([np.asarray(inputs["Wl1"], np.float32),
                           np.asarray(inputs["Wr1"], np.float32)], axis=1)
    wlr2 = np.concatenate([wl2, wr2], axis=1)
    shared = dict(
        wlr1=pad_kT(wlr1, 2 * HID),
        wlr2=two_chunk(wlr2),
        attrep1=attrep1,
        blr1=np.concatenate([rep_bias(inputs["bl1"]), rep_bias(inputs["br1"])], 1),
        bias1=rep_bias(inputs["bias1"]),
        blr2=np.concatenate([rep_bias(bl2), rep_bias(br2)], 1),
        bias2=rep_bias(bias2),
        inva2=np.broadcast_to(inva2[None, :], (P, HID)).copy(),
        hw1a=hW1[0:128].astype(np.float16),
        hw1b=hW1[128:256].astype(np.float16),
        hw1c=np.vstack([hW1[256:296], np.zeros((8, 128), np.float32)]).astype(np.float16),
        hw2=np.asarray(inputs["hW2"], np.float32).astype(np.float16),
        hw3=np.asarray(inputs["hW3"], np.float32).astype(np.float16),
        hb1=np.asarray(inputs["hb1"], np.float32).reshape(P, 1),
        hb2=np.asarray(inputs["hb2"], np.float32).reshape(64, 1),
        hb3=np.asarray(inputs["hb3"], np.float32).reshape(1, 1),
        ident=np.eye(P, dtype=np.float16),
    )

    # ---- per-core x slices, transposed + padded
    for c in range(NCORES):
        sel = core_of_node == c
        nodes = np.nonzero(sel)[0]
        slots = slot_of_node[nodes]
        xp = np.zeros((KPAD, NLOC), np.float32)
        xp[:IN_DIM, slots] = x[nodes].T
        per_core[c]["xt"] = xp.reshape(KCH, P, NLOC).transpose(1, 0, 2).reshape(
            P, KCH * NLOC).astype(np.float16)

    # ---- MLP batch assignment
    vcore = core_of_node[var_idx]
    vloc = slot_of_node[var_idx]
    batch_rows = []
    for c in range(NCORES):
        rows = np.nonzero(vcore == c)[0]
        assert len(rows) <= BLOC, f"core {c} has {len(rows)} batch rows > {BLOC}"
        batch_rows.append(rows)
        vi = np.zeros((1, BLOC), np.int64)
        vi[0, :len(rows)] = vloc[rows]
        per_core[c]["varloc"] = _wrap16(vi)
        wm = np.zeros((40, BLOC), np.float32)
        wm[:20, :len(rows)] = wt[rows].T
        wm[20:, :len(rows)] = mut[rows].T
        per_core[c]["wtmut"] = wm.astype(np.float16)

    return per_core, shared, batch_rows, (ew, n2pos)


def _build(key, no_collectives=False):
    ew, n2pos = key
    TG = ew // P
    T = TG + 1
    nc = bacc.Bacc("TRN2", target_bir_lowering=False, debug=False,
                   num_devices=1 if no_collectives else NCORES,
                   num_swdge_queues=4)

    io = {}
    io["xt"] = nc.dram_tensor("xt", [P, KCH * NLOC], f16, kind="ExternalInput")
    for nm, sh, dt in (
        ("wlr1", [P, KCH * 2 * HID], f16), ("wlr2", [P, 4 * HID], f16),
        ("attrep1", [P, T * HID], f16),
        ("blr1", [P, 2 * HID], f32), ("bias1", [P, HID], f32),
        ("blr2", [P, 2 * HID], f32), ("bias2", [P, HID], f32),
        ("inva2", [P, HID], f32),
        ("hw1a", [P, P], f16), ("hw1b", [P, P], f16), ("hw1c", [48, P], f16),
        ("hw2", [P, 64], f16), ("hw3", [64, 1], f16),
        ("hb1", [P, 1], f32), ("hb2", [64, 1], f32), ("hb3", [1, 1], f32),
        ("si", [WIN * P, ew // 16], i16),
        ("s_t", [WIN * P, T * P], fp8), ("s_T", [WIN * P, T * P], fp8),
        ("varloc", [P, BLOC // 16], i16), ("wtmut", [40, BLOC], f16),
        ("ident", [P, P], f16),
    ):
        io[nm] = nc.dram_tensor(nm, sh, dt, kind="ExternalInput")
    out = nc.dram_tensor("out", [1, BLOC], f32, kind="ExternalOutput")

    with tile.TileContext(nc) as tc:
        with (
            tc.tile_pool(name="const", bufs=1) as cp,
            tc.tile_pool(name="dram", bufs=1, space="DRAM") as dr,
        ):
            c_ = {}
            for nm in ("wlr2", "attrep1", "bias1", "blr2", "bias2", "inva2",
                       "hw1a", "hw1b", "hw1c", "hw2",
                       "hw3", "hb1", "hb2", "hb3", "varloc", "wtmut", "ident"):
                h = io[nm]
                c_[nm] = cp.tile(list(h.shape), h.dtype, tag=nm, name=f"c_{nm}")
                nc.sync.dma_start(c_[nm][:], h[:])

            xl1_loc = dr.tile([NLOC, HID], f16, name="xl1_loc")
            xr1_loc = dr.tile([NLOC, HID], f16)
            xl1_all = dr.tile([NLOC * NCORES, HID], f16, addr_space="Shared",
                              name="xl1_all")
            xl2_loc = dr.tile([NLOC, HID], f16, name="xl2_loc")
            xr2_loc = dr.tile([NLOC, HID], f16)
            xl2_all = dr.tile([NLOC * NCORES, HID], f16, addr_space="Shared",
                              name="xl2_all")
            h2_loc = dr.tile([NLOC, HID], f16)

            # ---------- phase A: layer-1 node transform ----------
            with (
                tc.tile_pool(name="pa_sb", bufs=2) as sb,
                tc.tile_pool(name="pa_xt", bufs=1) as xp,
                tc.tile_pool(name="pa_ps", bufs=4, space="PSUM") as ps,
            ):
                xt = xp.tile([P, KCH, NLOC], f16)
                xtv = io["xt"][:].rearrange("p (k n) -> p k n", k=KCH)
                for xq in range(4):
                    n0, n1 = xq * (NLOC // 4), (xq + 1) * (NLOC // 4)
                    nc.sync.dma_start(xt[:, :, n0:n1], xtv[:, :, n0:n1])
                wlr1 = xp.tile([P, KCH, 2 * HID], f16)
                nc.sync.dma_start(wlr1[:], io["wlr1"][:].rearrange("p (k n) -> p k n", k=KCH))
                blr1 = xp.tile([P, 2 * HID], f32)
                nc.sync.dma_start(blr1[:], io["blr1"][:])
                for nt in range(WIN):
                    pa = ps.tile([P, 2 * HID], f32, tag="pa")
                    for k in range(KCH):
                        nc.tensor.matmul(pa[:], lhsT=xt[:, k, nt * P:(nt + 1) * P],
                                         rhs=wlr1[:, k, :],
                                         start=(k == 0), stop=(k == KCH - 1))
                    o = sb.tile([P, 2 * HID], f16, tag="pao")
                    nc.vector.tensor_tensor(out=o[:], in0=pa[:], in1=blr1[:],
                                            op=OP.add)
                    rr = nt * P
                    nc.scalar.dma_start(xl1_loc[rr:rr + P, :], o[:, 0:HID])
                    nc.scalar.dma_start(xr1_loc[rr:rr + P, :], o[:, HID:2 * HID])

            nc.gpsimd.collective_compute(
                "AllGather", OP.bypass, replica_groups=[list(range(NCORES))],
                ins=[xl1_loc[:].opt()], outs=[xl1_all[:].opt()])

            # layer-1 message passing with the layer-2 node transform
            # pipelined per window (phase B)
            with (
                tc.tile_pool(name="pb_sb", bufs=2) as pbsb,
                tc.tile_pool(name="pb_ht", bufs=2) as pbhp,
                tc.tile_pool(name="pb_ps", bufs=2, space="PSUM") as pbps,
            ):
                blr2 = c_["blr2"]

                def post_l1(nt, h_t):
                    ht = pbhp.tile([P, 2, P], f16, tag="ht")
                    for k in range(2):
                        nc.sync.dma_start_transpose(
                            ht[:, k, :], h_t[:, k * P:(k + 1) * P])
                    pa = pbps.tile([P, 2 * HID], f32, tag="pb")
                    for k in range(2):
                        nc.tensor.matmul(
                            pa[:], lhsT=ht[:, k, :],
                            rhs=c_["wlr2"][:, k * 2 * HID:(k + 1) * 2 * HID],
                            start=(k == 0), stop=(k == 1))
                    o = pbsb.tile([P, 2 * HID], f16, tag="pbo")
                    nc.vector.tensor_tensor(out=o[:], in0=pa[:], in1=blr2[:],
                                            op=OP.add)
                    rr = nt * P
                    nc.scalar.dma_start(xl2_loc[rr:rr + P, :], o[:, 0:HID])
                    nc.scalar.dma_start(xr2_loc[rr:rr + P, :], o[:, HID:2 * HID])

                _emit_layer(nc, tc, ew=ew, heads=HEADS1, xl_all=xl1_all,
                            xl_loc=xl1_loc, xr_loc=xr1_loc, h_out=None,
                            attrep=c_["attrep1"], bias_mat=c_["bias1"],
                            inva=None, n2pos=None, io=io, ident=c_["ident"],
                            tag="l1", post_window=post_l1)

            nc.gpsimd.collective_compute(
                "AllGather", OP.bypass, replica_groups=[list(range(NCORES))],
                ins=[xl2_loc[:].opt()], outs=[xl2_all[:].opt()])

            _emit_layer(nc, tc, ew=ew, heads=1, xl_all=xl2_all,
                        xl_loc=xl2_loc, xr_loc=xr2_loc, h_out=h2_loc,
                        attrep=None, bias_mat=c_["bias2"], inva=c_["inva2"],
                        n2pos=n2pos, io=io, ident=c_["ident"], tag="l2")

            # ---------- MLP head ----------
            with (
                tc.tile_pool(name="mlp_sb", bufs=2) as sb,
                tc.tile_pool(name="mlp_ps", bufs=2, space="PSUM") as ps,
            ):
                sel = sb.tile([P, 2, BLOC], f16)
                nc.gpsimd.dma_gather(sel[:], h2_loc[:], c_["varloc"][:],
                                     num_idxs=BLOC, num_idxs_reg=BLOC,
                                     elem_size=HID, transpose=True)
                for c0, cn in ((0, 512), (512, BLOC - 512)):
                    z1p = ps.tile([P, 512], f32, tag="z1p")
                    nc.tensor.matmul(z1p[:, :cn], lhsT=c_["hw1a"][:],
                                     rhs=sel[:, 0, c0:c0 + cn], start=True, stop=False)
                    nc.tensor.matmul(z1p[:, :cn], lhsT=c_["hw1b"][:],
                                     rhs=sel[:, 1, c0:c0 + cn], start=False, stop=False)
                    nc.tensor.matmul(z1p[:, :cn], lhsT=c_["hw1c"][0:40, :],
                                     rhs=c_["wtmut"][:, c0:c0 + cn], start=False, stop=True)
                    z1 = sb.tile([P, 512], f16, tag="z1")
                    nc.scalar.activation(z1[:, :cn], z1p[:, :cn], AF.Relu,
                                         bias=c_["hb1"][:])
                    z2p = ps.tile([64, 512], f32, tag="z2p")
                    nc.tensor.matmul(z2p[:, :cn], lhsT=c_["hw2"][:],
                                     rhs=z1[:, :cn], start=True, stop=True)
                    z2 = sb.tile([64, 512], f16, tag="z2")
                    nc.scalar.activation(z2[:, :cn], z2p[:, :cn], AF.Relu,
                                         bias=c_["hb2"][:])
                    z3p = ps.tile([1, 512], f32, tag="z3p")
                    nc.tensor.matmul(z3p[:, :cn], lhsT=c_["hw3"][:],
                                     rhs=z2[:, :cn], start=True, stop=True)
                    z3 = sb.tile([1, 512], f32, tag="z3")
                    nc.scalar.activation(z3[:, :cn], z3p[:, :cn], AF.Identity,
                                         bias=c_["hb3"][:])
                    nc.sync.dma_start(out[0:1, c0:c0 + cn], z3[:, :cn])

    nc.compile()
    return nc


def _emit_layer(nc, tc, *, ew, heads, xl_all, xl_loc, xr_loc, h_out, attrep,
                bias_mat, inva, n2pos, io, ident, tag, post_window=None):
    TG = ew // P
    T = TG + 1
    CW = HID // heads
    NB = (T + 3) // 4
    # gather tiles split into 4 pieces <= 4 tiles (512 idxs) each
    npieces = (TG + 3) // 4
    bnds = [TG * i // npieces for i in range(npieces + 1)]
    pieces = [(a, b) for a, b in zip(bnds, bnds[1:]) if b > a]
    r_q = {}
    for tb, te in pieces:
        n = (te - tb) * P
        if n not in r_q:
            r_q[n] = nc.gpsimd.to_reg(n)
    with (
        tc.tile_pool(name=f"{tag}_g", bufs=6) as gp,
        tc.tile_pool(name=f"{tag}_s", bufs=3) as sp,
        tc.tile_pool(name=f"{tag}_si", bufs=8) as sip,
        tc.tile_pool(name=f"{tag}_lr", bufs=2) as lrp,
        tc.tile_pool(name=f"{tag}_gw", bufs=2) as gwp,
        tc.tile_pool(name=f"{tag}_wr", bufs=2) as wrp,
        tc.tile_pool(name=f"{tag}_e", bufs=2) as ep,
        tc.tile_pool(name=f"{tag}_pm", bufs=2, space="PSUM") as pmp,
        tc.tile_pool(name=f"{tag}_pa", bufs=2, space="PSUM") as pap,
    ):
        for w in range(WIN):
            rows = slice(w * P, (w + 1) * P)
            si = sip.tile([P, ew // 16], i16, tag="si")
            nc.sync.dma_start(si[:], io["si"][rows, :])
            sT = sp.tile([P, T, P], fp8, tag="sT")
            nc.sync.dma_start(sT[:], io["s_T"][rows, :].rearrange("p (t e) -> p t e", t=T))
            st = sp.tile([P, T, P], fp8, tag="st")
            nc.sync.dma_start(st[:], io["s_t"][rows, :].rearrange("p (t e) -> p t e", t=T))
            xrw = sp.tile([P, HID], f16, tag="xrw")
            nc.sync.dma_start(xrw[:], xr_loc[rows, :])

            # tile 0 = self loops: contiguous local rows, plain DMA
            gec = gp.tile([P, T, HID], f16, tag="gec")
            nc.sync.dma_start(gec[:, 0, :], xl_loc[rows, :])
            for j, (tb, te) in enumerate(pieces):
                n = (te - tb) * P
                nc.gpsimd.dma_gather(gec[:, 1 + tb:1 + te, :], xl_all[:],
                                     si[:, tb * 8:te * 8],
                                     num_idxs=n, num_idxs_reg=r_q[n],
                                     elem_size=HID, transpose=False,
                                     single_packet=False,
                                     queue_num=(w + j) % 4)

            # m = xl[src] + xr[dst] in PSUM (xr via s_T matmul, xl via identity
            # matmul); lr = leaky_relu(m)   (edge, channel)
            lr = lrp.tile([P, T, HID], f16, tag="lr")
            for b in range(NB):
                nb = min(4, T - 4 * b)
                pm = pmp.tile([P, 4, HID], f32, tag="pm")
                for tt in range(0, nb, 2):
                    nt2 = min(2, nb - tt)
                    nc.tensor.matmul(pm[:, tt:tt + nt2, :], lhsT=ident[:],
                                     rhs=gec[:, 4 * b + tt:4 * b + tt + nt2, :],
                                     start=True, stop=False)
                for tt in range(nb):
                    nc.tensor.matmul(pm[:, tt, :], lhsT=sT[:, 4 * b + tt, :],
                                     rhs=xrw[:], start=False, stop=True)
                if n2pos is None:
                    nc.scalar.activation(lr[:, 4 * b:4 * b + nb, :], pm[:, 0:nb, :],
                                         AF.Prelu, alpha=NEG)
                else:
                    # signed att2 folded into the channels: pos-att columns
                    # get Prelu(.2); neg-att columns get .2*Prelu(5).
                    if n2pos > 0:
                        nc.scalar.activation(lr[:, 4 * b:4 * b + nb, 0:n2pos],
                                             pm[:, 0:nb, 0:n2pos],
                                             AF.Prelu, alpha=NEG)
                    if n2pos < HID:
                        nc.scalar.activation(lr[:, 4 * b:4 * b + nb, n2pos:HID],
                                             pm[:, 0:nb, n2pos:HID],
                                             AF.Prelu, alpha=1.0 / NEG, scale=NEG)

            # logits per head: tree-sum of lr * att over each head's channel
            # block (L1); for L2 the att weights are folded in already.
            lra = lr[:].rearrange("p t (h c) -> p t h c", h=heads)
            if attrep is not None:
                nc.vector.tensor_tensor(
                    out=lr[:].rearrange("p t c -> p (t c)"),
                    in0=lr[:].rearrange("p t c -> p (t c)"),
                    in1=attrep[:], op=OP.mult)
            wdt = CW
            while wdt > 1:
                half = wdt // 2
                nc.vector.tensor_tensor(
                    out=lra[:, :, :, 0:half], in0=lra[:, :, :, 0:half],
                    in1=lra[:, :, :, half:wdt], op=OP.add)
                wdt = half

            # exp(logit) -> gwx tail (denominator) + replicated to all CW
            # channels via ACT broadcast-read (keeps gw-mult in DVE 2x mode)
            gwx = gwp.tile([P, T, HID + heads], f16, tag="gwx")
            wrep = wrp.tile([P, T, HID], f16, tag="wrep")
            wre4 = wrep[:].rearrange("p t (h c) -> p t h c", h=heads)
            nc.scalar.activation(
                gwx[:, :, HID:HID + heads].rearrange("p t (h o) -> p t h o", o=1),
                lra[:, :, :, 0:1], AF.Exp)
            nc.scalar.activation(
                wre4[:], lra[:, :, :, 0:1].to_broadcast([P, T, heads, CW]),
                AF.Exp)
            nc.vector.tensor_tensor(
                out=gwx[:, :, 0:HID], in0=gec[:].rearrange("p t c -> p (t c)"),
                in1=wrep[:].rearrange("p t c -> p (t c)"), op=OP.mult)

            agg = pap.tile([P, HID + heads], f32, tag="agg")
            for t in range(T):
                nc.tensor.matmul(agg[:], lhsT=st[:, t, :], rhs=gwx[:, t, :],
                                 start=(t == 0), stop=(t == T - 1))

            # normalize + bias + ELU
            den = ep.tile([P, heads], f32, tag="den")
            nc.vector.tensor_scalar_add(den[:], agg[:, HID:HID + heads], 1e-16)
            rden = ep.tile([P, heads], f32, tag="rden")
            nc.vector.reciprocal(rden[:], den[:])
            hb = ep.tile([P, HID], f32, tag="hb")
            if inva is not None:
                # hb = (agg * rden) * inva  (rden is a per-partition scalar)
                nc.vector.scalar_tensor_tensor(out=hb[:], in0=agg[:, 0:HID],
                                               scalar=rden[:, 0:1], in1=inva[:],
                                               op0=OP.mult, op1=OP.mult)
            else:
                nc.vector.tensor_tensor(
                    out=hb[:].rearrange("p (h c) -> p h c", h=heads),
                    in0=agg[:, 0:HID].rearrange("p (h c) -> p h c", h=heads),
                    in1=rden[:].to_broadcast([P, heads, CW]), op=OP.mult)
            nc.vector.tensor_tensor(out=hb[:], in0=hb[:], in1=bias_mat[:], op=OP.add)
            # ELU: hb + r + exp(-r) - 1  with r = relu(-hb)
            r = ep.tile([P, HID], f32, tag="r")
            nc.scalar.activation(r[:], hb[:], AF.Relu, scale=-1.0)
            ex = ep.tile([P, HID], f32, tag="ex")
            nc.scalar.activation(ex[:], r[:], AF.Exp, scale=-1.0)
            t1 = ep.tile([P, HID], f32, tag="t1")
            nc.vector.tensor_tensor(out=t1[:], in0=hb[:], in1=r[:], op=OP.add)
            h_t = ep.tile([P, HID], f16, tag="h_t")
            nc.vector.scalar_tensor_tensor(out=h_t[:], in0=ex[:], scalar=-1.0,
                                           in1=t1[:], op0=OP.add, op1=OP.add)
            if h_out is not None:
                nc.scalar.dma_start(h_out[rows, :], h_t[:])
            if post_window is not None:
                post_window(w, h_t)


def kernel(**inputs):
    per_core, shared, batch_rows, key = _preprocess(inputs)

    if key not in _nc_cache:
        _nc_cache[key] = _build(key)
    nc = _nc_cache[key]

    in_maps = []
    for c in range(NCORES):
        m = dict(shared)
        m.update(per_core[c])
        in_maps.append({k: np.ascontiguousarray(v) for k, v in m.items()})

    res = run_bass_kernel_spmd(nc, in_maps, core_ids=list(range(NCORES)))

    B = len(np.asarray(inputs["var_node_idx"]))
    out = np.zeros((B,), np.float32)
    for c in range(NCORES):
        rows = batch_rows[c]
        out[rows] = res.results[c]["out"][0, :len(rows)]
    return out


# revision 35
# speedup vs baseline: 1.0949x; 1.0450x over previous
"""Trainium2 Bass kernel for the 2-layer GATv2 + MLP-head model (nn_GAT_21028159881586).

Strategy (8 NeuronCores, SPMD single NEFF):
  * Destination-block partitioning: node -> (core, window-slot) assignment is
    LOAD-BALANCED on in-degree (LPT binning) so every one of the 240 windows
    has nearly the mean edge count -> minimal edge-tile padding T.
  * Per layer: node transforms on the local slice, one full AllGather of xl,
    then per destination window of 128 dst nodes:
      - self-loop edges form tile 0 of the window, loaded by a single
        contiguous HWDGE DMA from the core-local xl slice (no gather cost),
      - remaining edges: dma_gather of xl rows in (edge, channel) layout,
        split into 4 pieces <=512 idxs rotating over the 4 SWDGE queues
        (the random 512B reads are HBM-latency-bound; queue spread matters),
      - xr side via PE: dense fp8 scatter matrices s_T (dst x edge) and
        s_t (edge x dst); m = s_T.T @ xr + I @ gec in PSUM, Prelu on ACT,
      - L1 logits: lr * att_rep (DVE 2x) + halving tree per head,
      - L2 logits: signed att2 folded into Wl2/Wr2 columns on the host with
        channels permuted pos-att-first, so the Prelu stage splits into
        Prelu(.2) / scaled Prelu(5) column ranges and the logit is a plain
        halving-tree sum (no DVE multiply),
      - exp on ACT, replicated to all channels by an ACT broadcast-read exp
        (keeps the gw multiply in DVE 2x packed mode),
      - one PE matmul per 128-edge tile accumulates [agg | softmax-denom],
      - normalize (+ undo the att2 column scaling for L2), bias, ELU.
  * Layer-2 node transform pipelined per window, transposing h via SBUF-source
    DMA-transpose (no DRAM round-trip).
  * MLP head: batch rows are assigned to the core owning their var node.

fp16 data, fp32 PSUM accumulation.
"""

import heapq

import numpy as np

import concourse.bacc as bacc
import concourse.tile as tile
import concourse.mybir as mybir
from concourse.bass_utils import run_bass_kernel_spmd

fp8 = mybir.dt.float8e4

P = 128
NCORES = 8
N = 30000
WIN = 30
NLOC = WIN * P            # 3840
IN_DIM = 1281
KCH = 11
KPAD = KCH * P            # 1408
HID = 256
HEADS1 = 4
BLOC = 640
NEG = 0.2

f32 = mybir.dt.float32
f16 = mybir.dt.float16
i16 = mybir.dt.int16
AF = mybir.ActivationFunctionType
OP = mybir.AluOpType
AX = mybir.AxisListType

_nc_cache = {}


def _wrap16(idx2d: np.ndarray) -> np.ndarray:
    """(W, E) int -> (W*128, E//16) int16, wrapped in 16 partitions, replicated
    across the 8 gpsimd cores."""
    w, e = idx2d.shape
    assert e % 16 == 0
    t = idx2d.reshape(w, e // 16, 16).transpose(0, 2, 1)
    return np.tile(t, (1, 8, 1)).reshape(w * P, e // 16).astype(np.int16)


def _balance_nodes(deg: np.ndarray):
    """LPT binning of nodes into 240 (core, window) bins of <=128 nodes,
    balancing total in-degree per bin. Returns core_of_node, slot_of_node."""
    nbins = NCORES * WIN
    order = np.argsort(-deg, kind="stable")
    heap = [(0, b) for b in range(nbins)]
    heapq.heapify(heap)
    counts = np.zeros(nbins, np.int64)
    core_of = np.zeros(N, np.int64)
    slot_of = np.zeros(N, np.int64)
    for n in order:
        spill = []
        while True:
            s, b = heapq.heappop(heap)
            if counts[b] < P:
                break
            spill.append((s, b))
        for it in spill:
            heapq.heappush(heap, it)
        c, w = b // WIN, b % WIN
        core_of[n] = c
        slot_of[n] = w * P + counts[b]
        counts[b] += 1
        heapq.heappush(heap, (s + int(deg[n]), b))
    return core_of, slot_of


def _preprocess(inputs):
    x = np.asarray(inputs["x"], np.float32)
    ei = np.asarray(inputs["edge_index"]).astype(np.int64)
    var_idx = np.asarray(inputs["var_node_idx"]).astype(np.int64)
    wt = np.asarray(inputs["wt_onehot"], np.float32)
    mut = np.asarray(inputs["mut_onehot"], np.float32)

    # real edges only; the self loops become tile 0 of each window
    src_n = ei[0]
    dst_n = ei[1]
    deg = np.bincount(dst_n, minlength=N) + 1    # +1: self loop
    core_of_node, slot_of_node = _balance_nodes(deg)

    src_pad_all = core_of_node[src_n] * NLOC + slot_of_node[src_n]
    dcore = core_of_node[dst_n]
    dslot = slot_of_node[dst_n]

    order = np.argsort(dslot + dcore * NLOC, kind="stable")
    src_pad = src_pad_all[order]
    core_of = dcore[order]
    dloc = dslot[order]
    win_of = dloc // P

    flat = core_of * WIN + win_of
    counts = np.bincount(flat, minlength=NCORES * WIN)
    TG = int((counts.max() + P - 1) // P)        # gather tiles
    T = TG + 1                                   # + self tile
    ew = TG * P

    per_core = []
    for c in range(NCORES):
        sel = core_of == c
        sp_c, dl_c, w_c = src_pad[sel], dloc[sel], win_of[sel]
        srcw = np.zeros((WIN, ew), np.int64)     # padding gathers row 0
        drlw = np.full((WIN, T * P), -1, np.int64)   # -1 => padding edge
        # tile 0: self loops at slot position
        csel = core_of_node == c
        slots_c = slot_of_node[np.nonzero(csel)[0]]
        drlw[slots_c // P, slots_c % P] = slots_c % P
        for w in range(WIN):
            m = w_c == w
            k = int(m.sum())
            o = np.argsort(sp_c[m], kind="stable")   # HBM locality
            srcw[w, :k] = sp_c[m][o]
            drlw[w, P:P + k] = dl_c[m][o] - w * P
        si = _wrap16(srcw)                       # (WIN*P, ew//16) i16
        # s_t[w*128+p, t*128+d] = 1 if drl[w, t*128+p] == d  (edge-part, dst)
        dr_pt = drlw.reshape(WIN, T, P).transpose(0, 2, 1)     # [w, p, t]
        s_t = (dr_pt[:, :, :, None] == np.arange(P)[None, None, None, :])
        s_t = s_t.reshape(WIN * P, T * P)
        # s_T[w*128+d, t*128+e] = 1 if drl[w, t*128+e] == d   (dst-part, edge)
        dr_te = drlw.reshape(WIN, T, P)                        # [w, t, e]
        s_T = (np.arange(P)[None, :, None, None] == dr_te[:, None, :, :])
        s_T = s_T.reshape(WIN, P, T * P)
        per_core.append(dict(si=si,
                     s_t=s_t.astype(np.float32).astype(mybir.dt.np(fp8)),
                     s_T=s_T.reshape(WIN * P, T * P).astype(np.float32).astype(mybir.dt.np(fp8))))

    # ---- shared weights / constants
    def pad_kT(w, m):
        wp = np.zeros((KPAD, m), np.float32)
        wp[:IN_DIM] = w
        return wp.reshape(KCH, P, m).transpose(1, 0, 2).reshape(P, KCH * m).astype(np.float16)

    def two_chunk(w):
        m = w.shape[1]
        return w.reshape(2, P, m).transpose(1, 0, 2).reshape(P, 2 * m).astype(np.float16)

    # L1: att replicated to full window width (contiguous DVE multiply)
    att1 = np.asarray(inputs["att1"], np.float32)           # (4, 64)
    attrep1 = np.broadcast_to(np.tile(att1.reshape(1, HID), (1, T)),
                              (P, T * HID)).copy().astype(np.float16)

    # L2: fold signed att2 into the Wl2/Wr2 columns; permute pos-att first.
    att2 = np.asarray(inputs["att2"], np.float32).reshape(HID)
    perm2 = np.argsort(att2 < 0, kind="stable")             # pos/zero first
    n2pos = int((att2 >= 0).sum())
    a2p = att2[perm2]                                       # signed scales
    a2safe = np.where(np.abs(a2p) < 1e-12, 1.0, a2p)
    inva2 = (1.0 / a2safe).astype(np.float32)

    def rep_bias(b):
        return np.broadcast_to(np.asarray(b, np.float32)[None, :], (P, HID)).copy()

    wl2 = np.asarray(inputs["Wl2"], np.float32)[:, perm2] * a2p[None, :]
    wr2 = np.asarray(inputs["Wr2"], np.float32)[:, perm2] * a2p[None, :]
    bl2 = np.asarray(inputs["bl2"], np.float32)[perm2] * a2p
    br2 = np.asarray(inputs["br2"], np.float32)[perm2] * a2p
    bias2 = np.asarray(inputs["bias2"], np.float32)[perm2]

    hW1 = np.asarray(inputs["hW1"], np.float32).copy()
    hW1[0:HID] = hW1[0:HID][perm2]                          # permuted h2 rows
    wlr1 = np.concatenate([np.asarray(inputs["Wl1"], np.float32),
                           np.asarray(inputs["Wr1"], np.float32)], axis=1)
    wlr2 = np.concatenate([wl2, wr2], axis=1)
    shared = dict(
        wlr1=pad_kT(wlr1, 2 * HID),
        wlr2=two_chunk(wlr2),
        attrep1=attrep1,
        blr1=np.concatenate([rep_bias(inputs["bl1"]), rep_bias(inputs["br1"])], 1),
        bias1=rep_bias(inputs["bias1"]),
        blr2=np.concatenate([rep_bias(bl2), rep_bias(br2)], 1),
        bias2=rep_bias(bias2),
        inva2=np.broadcast_to(inva2[None, :], (P, HID)).copy(),
        hw1a=hW1[0:128].astype(np.float16),
        hw1b=hW1[128:256].astype(np.float16),
        hw1c=np.vstack([hW1[256:296], np.zeros((8, 128), np.float32)]).astype(np.float16),
        hw2=np.asarray(inputs["hW2"], np.float32).astype(np.float16),
        hw3=np.asarray(inputs["hW3"], np.float32).astype(np.float16),
        hb1=np.asarray(inputs["hb1"], np.float32).reshape(P, 1),
        hb2=np.asarray(inputs["hb2"], np.float32).reshape(64, 1),
        hb3=np.asarray(inputs["hb3"], np.float32).reshape(1, 1),
        ident=np.eye(P, dtype=np.float16),
    )

    # ---- per-core x slices, transposed + padded
    for c in range(NCORES):
        sel = core_of_node == c
        nodes = np.nonzero(sel)[0]
        slots = slot_of_node[nodes]
        xp = np.zeros((KPAD, NLOC), np.float32)
        xp[:IN_DIM, slots] = x[nodes].T
        per_core[c]["xt"] = xp.reshape(KCH, P, NLOC).transpose(1, 0, 2).reshape(
            P, KCH * NLOC).astype(np.float16)

    # ---- MLP batch assignment
    vcore = core_of_node[var_idx]
    vloc = slot_of_node[var_idx]
    batch_rows = []
    for c in range(NCORES):
        rows = np.nonzero(vcore == c)[0]
        assert len(rows) <= BLOC, f"core {c} has {len(rows)} batch rows > {BLOC}"
        batch_rows.append(rows)
        vi = np.zeros((1, BLOC), np.int64)
        vi[0, :len(rows)] = vloc[rows]
        per_core[c]["varloc"] = _wrap16(vi)
        wm = np.zeros((40, BLOC), np.float32)
        wm[:20, :len(rows)] = wt[rows].T
        wm[20:, :len(rows)] = mut[rows].T
        per_core[c]["wtmut"] = wm.astype(np.float16)

    return per_core, shared, batch_rows, (ew, n2pos)


def _build(key, no_collectives=False):
    ew, n2pos = key
    TG = ew // P
    T = TG + 1
    nc = bacc.Bacc("TRN2", target_bir_lowering=False, debug=False,
                   num_devices=1 if no_collectives else NCORES,
                   num_swdge_queues=4)

    io = {}
    io["xt"] = nc.dram_tensor("xt", [P, KCH * NLOC], f16, kind="ExternalInput")
    for nm, sh, dt in (
        ("wlr1", [P, KCH * 2 * HID], f16), ("wlr2", [P, 4 * HID], f16),
        ("attrep1", [P, T * HID], f16),
        ("blr1", [P, 2 * HID], f32), ("bias1", [P, HID], f32),
        ("blr2", [P, 2 * HID], f32), ("bias2", [P, HID], f32),
        ("inva2", [P, HID], f32),
        ("hw1a", [P, P], f16), ("hw1b", [P, P], f16), ("hw1c", [48, P], f16),
        ("hw2", [P, 64], f16), ("hw3", [64, 1], f16),
        ("hb1", [P, 1], f32), ("hb2", [64, 1], f32), ("hb3", [1, 1], f32),
        ("si", [WIN * P, ew // 16], i16),
        ("s_t", [WIN * P, T * P], fp8), ("s_T", [WIN * P, T * P], fp8),
        ("varloc", [P, BLOC // 16], i16), ("wtmut", [40, BLOC], f16),
        ("ident", [P, P], f16),
    ):
        io[nm] = nc.dram_tensor(nm, sh, dt, kind="ExternalInput")
    out = nc.dram_tensor("out", [1, BLOC], f32, kind="ExternalOutput")

    with tile.TileContext(nc) as tc:
        with (
            tc.tile_pool(name="const", bufs=1) as cp,
            tc.tile_pool(name="dram", bufs=1, space="DRAM") as dr,
        ):
            c_ = {}
            for nm in ("wlr2", "attrep1", "bias1", "blr2", "bias2", "inva2",
                       "hw1a", "hw1b", "hw1c", "hw2",
                       "hw3", "hb1", "hb2", "hb3", "varloc", "wtmut", "ident"):
                h = io[nm]
                c_[nm] = cp.tile(list(h.shape), h.dtype, tag=nm, name=f"c_{nm}")
                nc.sync.dma_start(c_[nm][:], h[:])

            xl1_loc = dr.tile([NLOC, HID], f16, name="xl1_loc")
            xr1_loc = dr.tile([NLOC, HID], f16)
            xl1_all = dr.tile([NLOC * NCORES, HID], f16, addr_space="Shared",
                              name="xl1_all")
            xl2_loc = dr.tile([NLOC, HID], f16, name="xl2_loc")
            xr2_loc = dr.tile([NLOC, HID], f16)
            xl2_all = dr.tile([NLOC * NCORES, HID], f16, addr_space="Shared",
                              name="xl2_all")
            h2_loc = dr.tile([NLOC, HID], f16)

            # ---------- phase A: layer-1 node transform ----------
            with (
                tc.tile_pool(name="pa_sb", bufs=2) as sb,
                tc.tile_pool(name="pa_xt", bufs=1) as xp,
                tc.tile_pool(name="pa_ps", bufs=4, space="PSUM") as ps,
            ):
                xt = xp.tile([P, KCH, NLOC], f16)
                xtv = io["xt"][:].rearrange("p (k n) -> p k n", k=KCH)
                for xq in range(4):
                    n0, n1 = xq * (NLOC // 4), (xq + 1) * (NLOC // 4)
                    nc.sync.dma_start(xt[:, :, n0:n1], xtv[:, :, n0:n1])
                wlr1 = xp.tile([P, KCH, 2 * HID], f16)
                nc.sync.dma_start(wlr1[:], io["wlr1"][:].rearrange("p (k n) -> p k n", k=KCH))
                blr1 = xp.tile([P, 2 * HID], f32)
                nc.sync.dma_start(blr1[:], io["blr1"][:])
                for nt in range(WIN):
                    pa = ps.tile([P, 2 * HID], f32, tag="pa")
                    for k in range(KCH):
                        nc.tensor.matmul(pa[:], lhsT=xt[:, k, nt * P:(nt + 1) * P],
                                         rhs=wlr1[:, k, :],
                                         start=(k == 0), stop=(k == KCH - 1))
                    o = sb.tile([P, 2 * HID], f16, tag="pao")
                    nc.vector.tensor_tensor(out=o[:], in0=pa[:], in1=blr1[:],
                                            op=OP.add)
                    rr = nt * P
                    nc.scalar.dma_start(xl1_loc[rr:rr + P, :], o[:, 0:HID])
                    nc.scalar.dma_start(xr1_loc[rr:rr + P, :], o[:, HID:2 * HID])

            nc.gpsimd.collective_compute(
                "AllGather", OP.bypass, replica_groups=[list(range(NCORES))],
                ins=[xl1_loc[:].opt()], outs=[xl1_all[:].opt()])

            # layer-1 message passing with the layer-2 node transform
            # pipelined per window (phase B)
            with (
                tc.tile_pool(name="pb_sb", bufs=2) as pbsb,
                tc.tile_pool(name="pb_ht", bufs=2) as pbhp,
                tc.tile_pool(name="pb_ps", bufs=2, space="PSUM") as pbps,
            ):
                blr2 = c_["blr2"]

                def post_l1(nt, h_t):
                    ht = pbhp.tile([P, 2, P], f16, tag="ht")
                    for k in range(2):
                        nc.sync.dma_start_transpose(
                            ht[:, k, :], h_t[:, k * P:(k + 1) * P])
                    pa = pbps.tile([P, 2 * HID], f32, tag="pb")
                    for k in range(2):
                        nc.tensor.matmul(
                            pa[:], lhsT=ht[:, k, :],
                            rhs=c_["wlr2"][:, k * 2 * HID:(k + 1) * 2 * HID],
                            start=(k == 0), stop=(k == 1))
                    o = pbsb.tile([P, 2 * HID], f16, tag="pbo")
                    nc.vector.tensor_tensor(out=o[:], in0=pa[:], in1=blr2[:],
                                            op=OP.add)
                    rr = nt * P
                    nc.scalar.dma_start(xl2_loc[rr:rr + P, :], o[:, 0:HID])
                    nc.scalar.dma_start(xr2_loc[rr:rr + P, :], o[:, HID:2 * HID])

                _emit_layer(nc, tc, ew=ew, heads=HEADS1, xl_all=xl1_all,
                            xl_loc=xl1_loc, xr_loc=xr1_loc, h_out=None,
                            attrep=c_["attrep1"], bias_mat=c_["bias1"],
                            inva=None, n2pos=None, io=io, ident=c_["ident"],
                            tag="l1", post_window=post_l1)

            nc.gpsimd.collective_compute(
                "AllGather", OP.bypass, replica_groups=[list(range(NCORES))],
                ins=[xl2_loc[:].opt()], outs=[xl2_all[:].opt()])

            _emit_layer(nc, tc, ew=ew, heads=1, xl_all=xl2_all,
                        xl_loc=xl2_loc, xr_loc=xr2_loc, h_out=h2_loc,
                        attrep=None, bias_mat=c_["bias2"], inva=c_["inva2"],
                        n2pos=n2pos, io=io, ident=c_["ident"], tag="l2")

            # ---------- MLP head ----------
            with (
                tc.tile_pool(name="mlp_sb", bufs=2) as sb,
                tc.tile_pool(name="mlp_ps", bufs=2, space="PSUM") as ps,
            ):
                sel = sb.tile([P, 2, BLOC], f16)
                nc.gpsimd.dma_gather(sel[:], h2_loc[:], c_["varloc"][:],
                                     num_idxs=BLOC, num_idxs_reg=BLOC,
                                     elem_size=HID, transpose=True)
                for c0, cn in ((0, 512), (512, BLOC - 512)):
                    z1p = ps.tile([P, 512], f32, tag="z1p")
                    nc.tensor.matmul(z1p[:, :cn], lhsT=c_["hw1a"][:],
                                     rhs=sel[:, 0, c0:c0 + cn], start=True, stop=False)
                    nc.tensor.matmul(z1p[:, :cn], lhsT=c_["hw1b"][:],
                                     rhs=sel[:, 1, c0:c0 + cn], start=False, stop=False)
                    nc.tensor.matmul(z1p[:, :cn], lhsT=c_["hw1c"][0:40, :],
                                     rhs=c_["wtmut"][:, c0:c0 + cn], start=False, stop=True)
                    z1 = sb.tile([P, 512], f16, tag="z1")
                    nc.scalar.activation(z1[:, :cn], z1p[:, :cn], AF.Relu,
                                         bias=c_["hb1"][:])
                    z2p = ps.tile([64, 512], f32, tag="z2p")
                    nc.tensor.matmul(z2p[:, :cn], lhsT=c_["hw2"][:],
                                     rhs=z1[:, :cn], start=True, stop=True)
                    z2 = sb.tile([64, 512], f16, tag="z2")
                    nc.scalar.activation(z2[:, :cn], z2p[:, :cn], AF.Relu,
                                         bias=c_["hb2"][:])
                    z3p = ps.tile([1, 512], f32, tag="z3p")
                    nc.tensor.matmul(z3p[:, :cn], lhsT=c_["hw3"][:],
                                     rhs=z2[:, :cn], start=True, stop=True)
                    z3 = sb.tile([1, 512], f32, tag="z3")
                    nc.scalar.activation(z3[:, :cn], z3p[:, :cn], AF.Identity,
                                         bias=c_["hb3"][:])
                    nc.sync.dma_start(out[0:1, c0:c0 + cn], z3[:, :cn])

    nc.compile()
    return nc


def _emit_layer(nc, tc, *, ew, heads, xl_all, xl_loc, xr_loc, h_out, attrep,
                bias_mat, inva, n2pos, io, ident, tag, post_window=None):
    TG = ew // P
    T = TG + 1
    CW = HID // heads
    NB = (T + 3) // 4
    # gather tiles split into 4 pieces <= 4 tiles (512 idxs) each
    npieces = (TG + 3) // 4
    bnds = [TG * i // npieces for i in range(npieces + 1)]
    pieces = [(a, b) for a, b in zip(bnds, bnds[1:]) if b > a]
    r_q = {}
    for tb, te in pieces:
        n = (te - tb) * P
        if n not in r_q:
            r_q[n] = nc.gpsimd.to_reg(n)
    with (
        tc.tile_pool(name=f"{tag}_g", bufs=6) as gp,
        tc.tile_pool(name=f"{tag}_s", bufs=3) as sp,
        tc.tile_pool(name=f"{tag}_si", bufs=8) as sip,
        tc.tile_pool(name=f"{tag}_lr", bufs=3) as lrp,
        tc.tile_pool(name=f"{tag}_gw", bufs=3) as gwp,
        tc.tile_pool(name=f"{tag}_wr", bufs=3) as wrp,
        tc.tile_pool(name=f"{tag}_e", bufs=3) as ep,
        tc.tile_pool(name=f"{tag}_pm", bufs=2, space="PSUM") as pmp,
        tc.tile_pool(name=f"{tag}_pa", bufs=2, space="PSUM") as pap,
    ):
        for w in range(WIN):
            rows = slice(w * P, (w + 1) * P)
            si = sip.tile([P, ew // 16], i16, tag="si")
            nc.sync.dma_start(si[:], io["si"][rows, :])
            sT = sp.tile([P, T, P], fp8, tag="sT")
            nc.sync.dma_start(sT[:], io["s_T"][rows, :].rearrange("p (t e) -> p t e", t=T))
            st = sp.tile([P, T, P], fp8, tag="st")
            nc.sync.dma_start(st[:], io["s_t"][rows, :].rearrange("p (t e) -> p t e", t=T))
            xrw = sp.tile([P, HID], f16, tag="xrw")
            nc.sync.dma_start(xrw[:], xr_loc[rows, :])

            # tile 0 = self loops: contiguous local rows, plain DMA
            gec = gp.tile([P, T, HID], f16, tag="gec")
            nc.sync.dma_start(gec[:, 0, :], xl_loc[rows, :])
            for j, (tb, te) in enumerate(pieces):
                n = (te - tb) * P
                nc.gpsimd.dma_gather(gec[:, 1 + tb:1 + te, :], xl_all[:],
                                     si[:, tb * 8:te * 8],
                                     num_idxs=n, num_idxs_reg=r_q[n],
                                     elem_size=HID, transpose=False,
                                     single_packet=False,
                                     queue_num=(w + j) % 4)

            # m = xl[src] + xr[dst] in PSUM (xr via s_T matmul, xl via identity
            # matmul); lr = leaky_relu(m)   (edge, channel)
            lr = lrp.tile([P, T, HID], f16, tag="lr")
            for b in range(NB):
                nb = min(4, T - 4 * b)
                pm = pmp.tile([P, 4, HID], f32, tag="pm")
                for tt in range(0, nb, 2):
                    nt2 = min(2, nb - tt)
                    nc.tensor.matmul(pm[:, tt:tt + nt2, :], lhsT=ident[:],
                                     rhs=gec[:, 4 * b + tt:4 * b + tt + nt2, :],
                                     start=True, stop=False)
                for tt in range(nb):
                    nc.tensor.matmul(pm[:, tt, :], lhsT=sT[:, 4 * b + tt, :],
                                     rhs=xrw[:], start=False, stop=True)
                if n2pos is None:
                    nc.scalar.activation(lr[:, 4 * b:4 * b + nb, :], pm[:, 0:nb, :],
                                         AF.Prelu, alpha=NEG)
                else:
                    # signed att2 folded into the channels: pos-att columns
                    # get Prelu(.2); neg-att columns get .2*Prelu(5).
                    if n2pos > 0:
                        nc.scalar.activation(lr[:, 4 * b:4 * b + nb, 0:n2pos],
                                             pm[:, 0:nb, 0:n2pos],
                                             AF.Prelu, alpha=NEG)
                    if n2pos < HID:
                        nc.scalar.activation(lr[:, 4 * b:4 * b + nb, n2pos:HID],
                                             pm[:, 0:nb, n2pos:HID],
                                             AF.Prelu, alpha=1.0 / NEG, scale=NEG)

            # logits per head: tree-sum of lr * att over each head's channel
            # block (L1); for L2 the att weights are folded in already.
            lra = lr[:].rearrange("p t (h c) -> p t h c", h=heads)
            if attrep is not None:
                nc.vector.tensor_tensor(
                    out=lr[:].rearrange("p t c -> p (t c)"),
                    in0=lr[:].rearrange("p t c -> p (t c)"),
                    in1=attrep[:], op=OP.mult)
            wdt = CW
            while wdt > 1:
                half = wdt // 2
                nc.vector.tensor_tensor(
                    out=lra[:, :, :, 0:half], in0=lra[:, :, :, 0:half],
                    in1=lra[:, :, :, half:wdt], op=OP.add)
                wdt = half

            # exp(logit) -> gwx tail (denominator) + replicated to all CW
            # channels via ACT broadcast-read (keeps gw-mult in DVE 2x mode)
            gwx = gwp.tile([P, T, HID + heads], f16, tag="gwx")
            wrep = wrp.tile([P, T, HID], f16, tag="wrep")
            wre4 = wrep[:].rearrange("p t (h c) -> p t h c", h=heads)
            nc.scalar.activation(
                gwx[:, :, HID:HID + heads].rearrange("p t (h o) -> p t h o", o=1),
                lra[:, :, :, 0:1], AF.Exp)
            nc.scalar.activation(
                wre4[:], lra[:, :, :, 0:1].to_broadcast([P, T, heads, CW]),
                AF.Exp)
            nc.vector.tensor_tensor(
                out=gwx[:, :, 0:HID], in0=gec[:].rearrange("p t c -> p (t c)"),
                in1=wrep[:].rearrange("p t c -> p (t c)"), op=OP.mult)

            agg = pap.tile([P, HID + heads], f32, tag="agg")
            for t in range(T):
                nc.tensor.matmul(agg[:], lhsT=st[:, t, :], rhs=gwx[:, t, :],
                                 start=(t == 0), stop=(t == T - 1))

            # normalize + bias + ELU
            den = ep.tile([P, heads], f32, tag="den")
            nc.vector.tensor_scalar_add(den[:], agg[:, HID:HID + heads], 1e-16)
            rden = ep.tile([P, heads], f32, tag="rden")
            nc.vector.reciprocal(rden[:], den[:])
            hb = ep.tile([P, HID], f32, tag="hb")
            if inva is not None:
                # hb = (agg * rden) * inva  (rden is a per-partition scalar)
                nc.vector.scalar_tensor_tensor(out=hb[:], in0=agg[:, 0:HID],
                                               scalar=rden[:, 0:1], in1=inva[:],
                                               op0=OP.mult, op1=OP.mult)
            else:
                nc.vector.tensor_tensor(
                    out=hb[:].rearrange("p (h c) -> p h c", h=heads),
                    in0=agg[:, 0:HID].rearrange("p (h c) -> p h c", h=heads),
                    in1=rden[:].to_broadcast([P, heads, CW]), op=OP.mult)
            nc.vector.tensor_tensor(out=hb[:], in0=hb[:], in1=bias_mat[:], op=OP.add)
            # ELU: hb + r + exp(-r) - 1  with r = relu(-hb)
            r = ep.tile([P, HID], f32, tag="r")
            nc.scalar.activation(r[:], hb[:], AF.Relu, scale=-1.0)
            ex = ep.tile([P, HID], f32, tag="ex")
            nc.scalar.activation(ex[:], r[:], AF.Exp, scale=-1.0)
            t1 = ep.tile([P, HID], f32, tag="t1")
            nc.vector.tensor_tensor(out=t1[:], in0=hb[:], in1=r[:], op=OP.add)
            h_t = ep.tile([P, HID], f16, tag="h_t")
            nc.vector.scalar_tensor_tensor(out=h_t[:], in0=ex[:], scalar=-1.0,
                                           in1=t1[:], op0=OP.add, op1=OP.add)
            if h_out is not None:
                nc.scalar.dma_start(h_out[rows, :], h_t[:])
            if post_window is not None:
                post_window(w, h_t)


def kernel(**inputs):
    per_core, shared, batch_rows, key = _preprocess(inputs)

    if key not in _nc_cache:
        _nc_cache[key] = _build(key)
    nc = _nc_cache[key]

    in_maps = []
    for c in range(NCORES):
        m = dict(shared)
        m.update(per_core[c])
        in_maps.append({k: np.ascontiguousarray(v) for k, v in m.items()})

    res = run_bass_kernel_spmd(nc, in_maps, core_ids=list(range(NCORES)))

    B = len(np.asarray(inputs["var_node_idx"]))
    out = np.zeros((B,), np.float32)
    for c in range(NCORES):
        rows = batch_rows[c]
        out[rows] = res.results[c]["out"][0, :len(rows)]
    return out
